# revision 19
# baseline (speedup 1.0000x reference)
"""Trainium2 Bass kernel for nn_Decoder_45509473468804 (gnn_message_passing).

Mathematical simplification (exact, not approximate): the reference net ends
with LayerNorm over the last dim, whose size is OUT=1:

    mu  = mean(h, -1, keepdims=True)          # mean of ONE element == h, exact in fp
    var = mean((h - mu)**2, -1, keepdims=True)  # == 0, exact
    out = (h - mu) * rsqrt(var + eps) * gamma + beta
        = 0 * rsqrt(eps) * gamma + beta
        = beta                                  (exactly, in any fp arithmetic)

So the output is exactly `ln_beta` broadcast to [E, 1] for ANY values of
x_node / edge_index / W* / b* / ln_gamma. The edge gather + 4-layer MLP is
dead code; the memory-roofline-optimal kernel reads ln_beta on device and
broadcasts it into the output.

Sharding: edge dim split evenly across the 8 NeuronCores (pure data
parallelism over edges, no communication) — each core produces its
400,000-edge shard of the output.

Per-core Bass kernel:
  1. DMA ln_beta [1] (DRAM) -> SBUF [128, 1] via a 0-stride partition-
     broadcast read (same idiom as tile_layernorm_bwd's ln_scale load).
  2. One vector-engine tensor_copy with a 0-stride free-dim broadcast AP
     fills a [128, 3125] f32 SBUF tile with beta.
  3. One DMA writes the 1.6 MB tile to the DRAM output shard
     (128 descriptors x 12.5 KB -> near-peak HBM write bandwidth).
"""

import numpy as np

N_EDGES = 3_200_000
N_CORES = 8
E_SHARD = N_EDGES // N_CORES  # 400_000 edges per core
P = 128                       # SBUF partitions
F = E_SHARD // P              # 3125 f32 per partition

_cache = {}


# Output is written in column chunks: a smaller first chunk lets the first DMA
# start early; later chunks overlap the in-flight DMAs. Sizes keep DMA
# descriptors (4*width bytes per partition row) >= 2KB.
CHUNK_SIZES = [512, 1306, 1307]
assert sum(CHUNK_SIZES) == F


def _chunks():
    out, lo = [], 0
    for w in CHUNK_SIZES:
        out.append((lo, lo + w))
        lo += w
    return out


def _build_nc():
    import concourse.bass as bass
    import concourse.mybir as mybir

    nc = bass.Bass()
    beta_ext = nc.dram_tensor("ln_beta", [1], mybir.dt.float32, kind="ExternalInput")
    out_ext = nc.dram_tensor("out", [P, F], mybir.dt.float32, kind="ExternalOutput")

    cols = _chunks()
    n_dmas = len(cols)  # one per output chunk (beta arrives via reg_load)

    with (
        nc.sbuf_tensor([1, P], mybir.dt.float32) as ones_1P,
        nc.sbuf_tensor([1, 1], mybir.dt.float32) as beta_11,
        nc.sbuf_tensor([P, 1], mybir.dt.float32) as beta_P1,
        nc.sbuf_tensor([P, F], mybir.dt.float32) as tile,
        nc.psum_tensor([P, 1], mybir.dt.float32) as pbeta,
        nc.semaphore() as dma_sem,
        nc.semaphore() as bsem,  # beta landed in SBUF [1,1]
        nc.semaphore() as ssem,  # ones memset done
        nc.semaphore() as msem,  # matmul done
        nc.semaphore() as csem,  # beta_P1 (psum->sbuf) done
        nc.semaphore() as vfsem,  # vector fills done (one inc per chunk)
        nc.Block() as block,
    ):

        @block.gpsimd
        def _(gpsimd):
            # engine-register DRAM read: beta is in SBUF ~2us after the
            # engines start, vs ~3.5us for a DMA round-trip
            with gpsimd.register("rbeta") as rbeta:
                gpsimd.reg_load(rbeta, beta_ext[None, :].bitcast(mybir.dt.int32))
                gpsimd.reg_save(
                    beta_11[0:1, 0:1].bitcast(mybir.dt.int32), rbeta
                ).then_inc(bsem, 1)

        @block.tensor
        def _(tensor):
            # pbeta[p, 0] = sum_k ones[k, p] * beta[k, 0] = beta  (K=1):
            # broadcasts beta across all 128 partitions in one PE pass
            tensor.wait_ge(bsem, 1)
            tensor.wait_ge(ssem, 1)
            tensor.matmul(
                pbeta[:], ones_1P[:], beta_11[:], start=True, stop=True
            ).then_inc(msem, 1)

        @block.vector
        def _(vector):
            vector.memset(ones_1P[:], 1.0).then_inc(ssem, 1)
            vector.wait_ge(msem, 1)
            # PSUM broadcast reads are ~2x slower than SBUF; bounce to SBUF
            vector.tensor_copy(beta_P1[:], pbeta[:]).then_inc(csem, 1)
            vector.wait_ge(csem, 1)  # same-engine RAW hazard on beta_P1
            for lo, hi in cols:
                vector.tensor_copy(
                    tile[:, lo:hi], beta_P1[:].to_broadcast((P, hi - lo))
                ).then_inc(vfsem, 1)

        @block.sync
        def _(sync):
            for c, (lo, hi) in enumerate(cols):
                if c % 2 == 0:
                    sync.wait_ge(vfsem, c + 1)
                    sync.dma_start(out=out_ext[:, lo:hi], in_=tile[:, lo:hi]).then_inc(
                        dma_sem, 16
                    )
            sync.wait_ge(dma_sem, 16 * n_dmas)

        @block.scalar
        def _(scalar):
            # second HW-DGE ring for the odd chunks (no compute on scalar)
            for c, (lo, hi) in enumerate(cols):
                if c % 2 == 1:
                    scalar.wait_ge(vfsem, c + 1)
                    scalar.dma_start(
                        out=out_ext[:, lo:hi], in_=tile[:, lo:hi]
                    ).then_inc(dma_sem, 16)

    return nc


def _ensure_ntff_hook():
    """The image's `antenv` package lacks the `axon_hooks` shim that
    bass_utils' trace path imports; build it from trn_agent_boot's ctypes
    hook so neuron-profile NTFF capture works under axon."""
    import sys, types

    if "antenv.axon_hooks" in sys.modules:
        return
    try:
        import importlib.util

        if importlib.util.find_spec("antenv.axon_hooks") is not None:
            return
    except (ImportError, ModuleNotFoundError):
        pass
    try:
        from trn_agent_boot.trn_boot import _ntff_profile_via_ctypes

        hook = _ntff_profile_via_ctypes("/opt/axon/libaxon_pjrt.so")
    except Exception:
        hook = None
    mod = types.ModuleType("antenv.axon_hooks")
    mod.get_axon_ntff_profile_hook = lambda: hook
    mod.set_axon_ntff_profile_hook = lambda h: None
    sys.modules["antenv.axon_hooks"] = mod


def _run(ln_beta: np.ndarray, trace: bool = False, **kwargs):
    """Compile (cached) + run the SPMD kernel on cores 0-7. Returns
    BassKernelResults; .results[i]["out"] is core i's [128, 3125] shard."""
    from concourse.bass_utils import run_bass_kernel_spmd

    if trace:
        _ensure_ntff_hook()

    if "nc" not in _cache:
        _cache["nc"] = _build_nc()
    nc = _cache["nc"]

    core_ids = list(range(N_CORES))
    in_maps = [{"ln_beta": ln_beta.copy()} for _ in core_ids]
    return run_bass_kernel_spmd(nc, in_maps, core_ids, trace=trace, **kwargs)


def kernel(**inputs: np.ndarray) -> np.ndarray:
    ln_beta = np.ascontiguousarray(
        np.asarray(inputs["ln_beta"], dtype=np.float32)
    ).reshape([1])

    res = _run(ln_beta)
    shards = [np.asarray(r["out"]).reshape(E_SHARD, 1) for r in res.results]
    return np.concatenate(shards, axis=0)
